# revision 9
# baseline (speedup 1.0000x reference)
"""Distributed inverse real vector SHT on 8 Trainium2 NeuronCores.

Decomposition (2D polar x azimuth, per the original model's parallelism):
  Stage 1 (sharded over m): for each m, the four Legendre contractions are
  two accumulating matmuls  Z[m] = X1[m]^T @ dT0[m] + X2[m]^T @ dT1[m]
  where the 128 columns of X1/X2 pack the four (re/im x s/t) input blocks
  with signs arranged so the PSUM accumulation directly produces
  rows [srl, sim, tim, trl] (no separate combine step).
  All-to-all (split 70/30 in m so it overlaps stage 1's tail): re-shard
  from m-split to nlat(k)-split.
  Stage 2 (sharded over k): transpose Z on the PE (c2 partition -> m
  partition), then the irfft is a real matmul against precomputed cos/sin
  tables contracting over (re/im, m).
"""
import sys
import os
sys.path.insert(0, '/opt/trn_rl_repo')
import numpy as np
import ml_dtypes

from concourse import bacc, tile, mybir, masks
from concourse.bass_utils import run_bass_kernel_spmd

B, C, L, M, K, N = 1, 32, 361, 361, 361, 720
NC = 8
MP = 368                    # m padded to 8*46
MC = MP // NC               # 46 m's per core
MA, MB = 32, 14             # per-core m split: A (first 32), B (last 14)
KC = 46                     # k's per core
KG = 48                     # padded k width in the output staging (12*4)
KPP = MP                    # 368: k padded total in stage-1 tables
LP = 384                    # l padded to 3*128
LCH = 3
NG = KG // 4                # 12 kj-groups per core
BF16 = ml_dtypes.bfloat16

_CACHE = {}


def _build():
    nc = bacc.Bacc("TRN2", target_bir_lowering=False, debug=False,
                   num_devices=NC)
    xsh = nc.dram_tensor("xsh", [128, MC, 2, LCH, 128], mybir.dt.bfloat16,
                         kind="ExternalInput")
    dsh = nc.dram_tensor("dsh", [128, MC, 2, LCH, KPP], mybir.dt.bfloat16,
                         kind="ExternalInput")
    ctab = nc.dram_tensor("ctab", [128, 2, LCH, N], mybir.dt.bfloat16,
                          kind="ExternalInput")
    outsh = nc.dram_tensor("outsh", [2, C, KG, N], mybir.dt.float32,
                           kind="ExternalOutput")

    m_blocks = []
    m0 = 0
    while m0 < MC:
        cnt = min(8, MC - m0)
        m_blocks.append((m0, cnt))
        m0 += cnt

    with tile.TileContext(nc) as tc:
        with tc.tile_pool(name="dram", bufs=1, space="DRAM") as dram, \
             tc.tile_pool(name="const", bufs=1) as constp:
            a2aA_in = dram.tile([NC, 128, MA, KC], mybir.dt.bfloat16)
            a2aA_out = dram.tile([NC, 128, MA, KC], mybir.dt.bfloat16)
            a2aB_in = dram.tile([NC, 128, MB, KC], mybir.dt.bfloat16)
            a2aB_out = dram.tile([NC, 128, MB, KC], mybir.dt.bfloat16)
            ident = constp.tile([128, 128], mybir.dt.bfloat16)
            masks.make_identity(nc, ident[:])

            # ---------------- stage 1: Legendre contractions (m-sharded)
            with tc.tile_pool(name="s1", bufs=2) as s1, \
                 tc.tile_pool(name="zs", bufs=1) as zs, \
                 tc.tile_pool(name="ps1", bufs=4, space="PSUM") as ps1:
                # kg-major staging so each a2a write is contiguous on both
                # the SBUF and the DRAM side
                zstA = zs.tile([128, NC, MA, KC], mybir.dt.bfloat16,
                               tag="zstA")
                zstB = zs.tile([128, NC, MB, KC], mybir.dt.bfloat16,
                               tag="zstB")
                for (m0, cnt) in m_blocks:
                    dt = s1.tile([128, 8, 2, LCH, KPP], mybir.dt.bfloat16,
                                 tag="dt")
                    xt = s1.tile([128, 8, 2, LCH, 128], mybir.dt.bfloat16,
                                 tag="xt")
                    nc.sync.dma_start(out=dt[:, :cnt], in_=dsh[:, m0:m0+cnt])
                    nc.sync.dma_start(out=xt[:, :cnt], in_=xsh[:, m0:m0+cnt])
                    for ml in range(cnt):
                        zt = ps1.tile([128, KPP], mybir.dt.float32, tag="zt")
                        for lc in range(LCH):
                            for w in range(2):
                                nc.tensor.matmul(
                                    out=zt[:],
                                    lhsT=xt[:, ml, w, lc, :],
                                    rhs=dt[:, ml, w, lc, :],
                                    start=(lc == 0 and w == 0),
                                    stop=(lc == LCH - 1 and w == 1),
                                )
                        mg = m0 + ml
                        dst = (zstA[:, :, mg, :] if mg < MA
                               else zstB[:, :, mg - MA, :])
                        nc.vector.tensor_copy(
                            out=dst,
                            in_=zt[:].rearrange("p (g k) -> p g k", k=KC))
                    # a2a writes go on the scalar HWDGE ring so they start
                    # as soon as their half is complete
                    if m0 + cnt == MA:
                        for kg in range(NC):
                            nc.scalar.dma_start(
                                out=a2aA_in[kg], in_=zstA[:, kg])
                for kg in range(NC):
                    nc.scalar.dma_start(out=a2aB_in[kg], in_=zstB[:, kg])

            nc.gpsimd.collective_compute(
                "AllToAll", mybir.AluOpType.bypass,
                replica_groups=[list(range(NC))],
                ins=[a2aA_in.opt()], outs=[a2aA_out.opt()],
            )
            nc.gpsimd.collective_compute(
                "AllToAll", mybir.AluOpType.bypass,
                replica_groups=[list(range(NC))],
                ins=[a2aB_in.opt()], outs=[a2aB_out.opt()],
            )

            # ---------------- stage 2: irfft as matmul (k-sharded)
            with tc.tile_pool(name="s2", bufs=1) as s2, \
                 tc.tile_pool(name="ob", bufs=3) as ob, \
                 tc.tile_pool(name="ps2po", bufs=2, space="PSUM") as ps2po, \
                 tc.tile_pool(name="ps2tp", bufs=4, space="PSUM") as ps2tp:
                ct = s2.tile([128, 2, LCH, N], mybir.dt.bfloat16, tag="ct")
                nc.sync.dma_start(out=ct[:], in_=ctab[:])
                ztmpA = s2.tile([128, NC * MA, KC], mybir.dt.bfloat16,
                                tag="ztmpA")
                ztmpB = s2.tile([128, NC * MB, KC], mybir.dt.bfloat16,
                                tag="ztmpB")
                nc.sync.dma_start(
                    out=ztmpA.rearrange("c (b m) k -> c b m k", m=MA),
                    in_=a2aA_out.rearrange("b c m k -> c b m k"))
                nc.sync.dma_start(
                    out=ztmpB.rearrange("c (b m) k -> c b m k", m=MB),
                    in_=a2aB_out.rearrange("b c m k -> c b m k"))

                # on-chip transpose (c2 partitions -> m partitions), packed
                # as [m, b, g, kj, c] so each (b, g) slice is a contiguous
                # 128-wide stationary operand with rows (kj, c)
                mchunks = [(ztmpA, 0, 128), (ztmpA, 128, 128),
                           (ztmpB, 0, NC * MB)]
                ztr = []
                for mc, (src, off, mcnt) in enumerate(mchunks):
                    t = s2.tile([128, 4, NG, 4, 32], mybir.dt.bfloat16,
                                tag=f"ztr{mc}")
                    nc.vector.memset(t[:, :, NG-1, 2:4, :], 0.0)
                    for kj in range(KC):
                        # transpose c2->m via a regular matmul against the
                        # identity (runs warm, unlike PE transpose-mode)
                        pt = ps2tp.tile([128, 128], mybir.dt.float32,
                                        tag="pt")
                        nc.tensor.matmul(
                            out=pt[:mcnt], lhsT=src[:, off:off+mcnt, kj],
                            rhs=ident[:], start=True, stop=True)
                        nc.vector.tensor_copy(
                            out=t[:mcnt, :, kj // 4, kj % 4, :],
                            in_=pt[:mcnt])
                    ztr.append(t)

                # comp 0 (s): srl rows (b=0) w/ Cre, sim rows (b=1) w/ Cim
                # comp 1 (t): trl rows (b=3) w/ Cre, tim rows (b=2) w/ Cim
                comp_seq = [((0, 0), (1, 1)), ((3, 0), (2, 1))]
                for comp in range(2):
                    for g in range(NG):
                        po0 = ps2po.tile([128, 360], mybir.dt.float32,
                                         tag="po0")
                        po1 = ps2po.tile([128, 360], mybir.dt.float32,
                                         tag="po1")
                        i = 0
                        for (b, reim) in comp_seq[comp]:
                            for mc, (src, off, mcnt) in enumerate(mchunks):
                                lhsT = ztr[mc][:mcnt, b, g]
                                nc.tensor.matmul(
                                    out=po0[:], lhsT=lhsT,
                                    rhs=ct[:mcnt, reim, mc, 0:360],
                                    start=(i == 0), stop=(i == 5))
                                nc.tensor.matmul(
                                    out=po1[:], lhsT=lhsT,
                                    rhs=ct[:mcnt, reim, mc, 360:720],
                                    start=(i == 0), stop=(i == 5))
                                i += 1
                        osb = ob.tile([128, N], mybir.dt.float32, tag="osb")
                        nc.vector.tensor_copy(out=osb[:, 0:360], in_=po0[:])
                        nc.vector.tensor_copy(out=osb[:, 360:720], in_=po1[:])
                        # psum rows are (kj, c); alternate HWDGE rings
                        eng = nc.sync if (g % 2 == 0) else nc.scalar
                        eng.dma_start(
                            out=outsh[comp].rearrange(
                                "c k n -> k c n")[g*4:(g+1)*4],
                            in_=osb[:],
                        )
    nc.compile()
    return nc


def _m_perm():
    """Row order of the m axis as seen by stage 2 (A-half then B-half)."""
    perm = [mb * MC + ml for mb in range(NC) for ml in range(MA)]
    perm += [mb * MC + ml for mb in range(NC) for ml in range(MA, MC)]
    return np.array(perm)


def _host_prep(x_re, x_im, d0, d1):
    xr0, xr1 = x_re[0, :, 0], x_re[0, :, 1]   # (32, L, M)
    xi0, xi1 = x_im[0, :, 0], x_im[0, :, 1]

    def mkx(blocks):
        x = np.concatenate(blocks, axis=0)            # (128, L, M)
        x = np.transpose(x, (2, 1, 0))                # (M, L, 128)
        xp = np.zeros((MP, LP, 128), BF16)
        xp[:M, :L] = x
        return xp
    X1 = mkx([xr0, xi0, -xi1, -xr1])
    X2 = mkx([-xi1, xr1, xr0, -xi0])
    # xsh[core][p, ml, which, lc, c'] = X{which}[core*MC+ml, lc*128+p, c']
    xv = np.stack([X1, X2], axis=1)                   # (MP, 2, LP, 128)
    xv = xv.reshape(NC, MC, 2, LCH, 128, 128)         # (i, ml, w, lc, p, c)
    xv = np.ascontiguousarray(xv.transpose(0, 4, 1, 2, 3, 5))

    def mkd(d):
        dp = np.zeros((MP, LP, KPP), BF16)
        dp[:M, :L, :K] = np.transpose(d, (0, 2, 1))
        return dp
    D0, D1 = mkd(d0), mkd(d1)
    dv = np.stack([D0, D1], axis=1)                   # (MP, 2, LP, KPP)
    dv = dv.reshape(NC, MC, 2, LCH, 128, KPP)
    dv = np.ascontiguousarray(dv.transpose(0, 4, 1, 2, 3, 5))

    m = np.arange(MP, dtype=np.float64)[:, None]
    n = np.arange(N, dtype=np.float64)[None, :]
    th = 2.0 * np.pi * (m * n) / N
    w = np.full((MP, 1), 2.0); w[0] = 1.0; w[360] = 1.0; w[361:] = 0.0
    Cre = (w * np.cos(th)).astype(np.float32)
    Cim = (-w * np.sin(th)).astype(np.float32)
    Cim[0] = 0.0; Cim[360] = 0.0; Cim[361:] = 0.0
    cv = np.stack([Cre, Cim], axis=1)                 # (MP, 2, N)
    cv = cv[_m_perm()]                                # stage-2 m order
    cv = np.concatenate(
        [cv, np.zeros((LCH * 128 - MP, 2, N), np.float32)], axis=0)
    cv = cv.reshape(LCH, 128, 2, N)
    cv = np.ascontiguousarray(cv.transpose(1, 2, 0, 3)).astype(BF16)
    return xv, dv, cv


def kernel(x_re, x_im, d0, d1):
    if "nc" not in _CACHE:
        _CACHE["nc"] = _build()
    nc = _CACHE["nc"]

    xv, dv, cv = _host_prep(np.asarray(x_re), np.asarray(x_im),
                            np.asarray(d0), np.asarray(d1))
    in_maps = [{"xsh": xv[i], "dsh": dv[i], "ctab": cv} for i in range(NC)]
    res = run_bass_kernel_spmd(nc, in_maps, list(range(NC)))

    out = np.empty((B, C, 2, K, N), np.float32)
    for i in range(NC):
        k0 = i * KC
        k1 = min(K, k0 + KC)
        o = res.results[i]["outsh"]        # [2, C, KG, N]
        out[0, :, 0, k0:k1] = o[0, :, :k1-k0]
        out[0, :, 1, k0:k1] = o[1, :, :k1-k0]
    return out


# revision 13
# speedup vs baseline: 1.0130x; 1.0130x over previous
"""Distributed inverse real vector SHT on 8 Trainium2 NeuronCores.

Decomposition (2D polar x azimuth, per the original model's parallelism):
  Stage 1 (sharded over m): for each m, the four Legendre contractions are
  two accumulating matmuls  Z[m] = X1[m]^T @ dT0[m] + X2[m]^T @ dT1[m]
  where the 128 columns of X1/X2 pack the four (re/im x s/t) input blocks
  with signs arranged so the PSUM accumulation directly produces
  rows [srl, sim, tim, trl] (no separate combine step).
  All-to-all (split 70/30 in m so it overlaps stage 1's tail): re-shard
  from m-split to nlat(k)-split.
  Stage 2 (sharded over k): transpose Z on the PE (c2 partition -> m
  partition), then the irfft is a real matmul against precomputed cos/sin
  tables contracting over (re/im, m).
"""
import sys
import os
sys.path.insert(0, '/opt/trn_rl_repo')
import numpy as np
import ml_dtypes

from concourse import bacc, tile, mybir, masks
from concourse.bass_utils import run_bass_kernel_spmd

B, C, L, M, K, N = 1, 32, 361, 361, 361, 720
NC = 8
MP = 368                    # m padded to 8*46
MC = MP // NC               # 46 m's per core
MA, MB = 32, 14             # per-core m split: A (first 32), B (last 14)
KC = 46                     # k's per core
KG = 48                     # padded k width in the output staging (12*4)
KPP = MP                    # 368: k padded total in stage-1 tables
LP = 384                    # l padded to 3*128
LCH = 3
NG = KG // 4                # 12 kj-groups per core
BF16 = ml_dtypes.bfloat16

_CACHE = {}


def _build():
    nc = bacc.Bacc("TRN2", target_bir_lowering=False, debug=False,
                   num_devices=NC)
    xsh = nc.dram_tensor("xsh", [128, MC, 2, LCH, 128], mybir.dt.bfloat16,
                         kind="ExternalInput")
    dsh = nc.dram_tensor("dsh", [128, MC, 2, LCH, KPP], mybir.dt.bfloat16,
                         kind="ExternalInput")
    ctab = nc.dram_tensor("ctab", [128, 2, LCH, N], mybir.dt.bfloat16,
                          kind="ExternalInput")
    outsh = nc.dram_tensor("outsh", [2, C, KG, N], mybir.dt.float32,
                           kind="ExternalOutput")

    m_blocks = []
    m0 = 0
    while m0 < MC:
        cnt = min(8, MC - m0)
        m_blocks.append((m0, cnt))
        m0 += cnt

    with tile.TileContext(nc) as tc:
        with tc.tile_pool(name="dram", bufs=1, space="DRAM") as dram, \
             tc.tile_pool(name="const", bufs=1) as constp:
            a2aA_in = dram.tile([NC, 128, MA, KC], mybir.dt.bfloat16)
            a2aA_out = dram.tile([NC, 128, MA, KC], mybir.dt.bfloat16)
            a2aB_in = dram.tile([NC, 128, MB, KC], mybir.dt.bfloat16)
            a2aB_out = dram.tile([NC, 128, MB, KC], mybir.dt.bfloat16)
            ident = constp.tile([128, 128], mybir.dt.bfloat16)
            masks.make_identity(nc, ident[:])

            # ---------------- stage 1: Legendre contractions (m-sharded)
            with tc.tile_pool(name="s1", bufs=2) as s1, \
                 tc.tile_pool(name="zs", bufs=1) as zs, \
                 tc.tile_pool(name="ps1", bufs=4, space="PSUM") as ps1:
                # kg-major staging so each a2a write is contiguous on both
                # the SBUF and the DRAM side
                zstA = zs.tile([128, NC, MA, KC], mybir.dt.bfloat16,
                               tag="zstA")
                zstB = zs.tile([128, NC, MB, KC], mybir.dt.bfloat16,
                               tag="zstB")
                for (m0, cnt) in m_blocks:
                    dt = s1.tile([128, 8, 2, LCH, KPP], mybir.dt.bfloat16,
                                 tag="dt")
                    xt = s1.tile([128, 8, 2, LCH, 128], mybir.dt.bfloat16,
                                 tag="xt")
                    nc.sync.dma_start(out=dt[:, :cnt], in_=dsh[:, m0:m0+cnt])
                    nc.sync.dma_start(out=xt[:, :cnt], in_=xsh[:, m0:m0+cnt])
                    for ml in range(cnt):
                        zt = ps1.tile([128, KPP], mybir.dt.float32, tag="zt")
                        for lc in range(LCH):
                            for w in range(2):
                                nc.tensor.matmul(
                                    out=zt[:],
                                    lhsT=xt[:, ml, w, lc, :],
                                    rhs=dt[:, ml, w, lc, :],
                                    start=(lc == 0 and w == 0),
                                    stop=(lc == LCH - 1 and w == 1),
                                )
                        mg = m0 + ml
                        dst = (zstA[:, :, mg, :] if mg < MA
                               else zstB[:, :, mg - MA, :])
                        nc.vector.tensor_copy(
                            out=dst,
                            in_=zt[:].rearrange("p (g k) -> p g k", k=KC))
                    # a2a writes go on the scalar HWDGE ring so they start
                    # as soon as their half is complete
                    if m0 + cnt == MA:
                        for kg in range(NC):
                            nc.scalar.dma_start(
                                out=a2aA_in[kg], in_=zstA[:, kg])
                for kg in range(NC):
                    nc.scalar.dma_start(out=a2aB_in[kg], in_=zstB[:, kg])

            nc.gpsimd.collective_compute(
                "AllToAll", mybir.AluOpType.bypass,
                replica_groups=[list(range(NC))],
                ins=[a2aA_in.opt()], outs=[a2aA_out.opt()],
            )
            nc.gpsimd.collective_compute(
                "AllToAll", mybir.AluOpType.bypass,
                replica_groups=[list(range(NC))],
                ins=[a2aB_in.opt()], outs=[a2aB_out.opt()],
            )

            # ---------------- stage 2: irfft as matmul (k-sharded)
            with tc.tile_pool(name="s2", bufs=1) as s2, \
                 tc.tile_pool(name="ob", bufs=4) as ob:
                ct = s2.tile([128, 2, LCH, N], mybir.dt.bfloat16, tag="ct")
                nc.sync.dma_start(out=ct[:], in_=ctab[:])
                ztmpA = s2.tile([128, NC * MA, KC], mybir.dt.bfloat16,
                                tag="ztmpA")
                ztmpB = s2.tile([128, NC * MB, KC], mybir.dt.bfloat16,
                                tag="ztmpB")
                nc.sync.dma_start(
                    out=ztmpA.rearrange("c (b m) k -> c b m k", m=MA),
                    in_=a2aA_out.rearrange("b c m k -> c b m k"))
                nc.sync.dma_start(
                    out=ztmpB.rearrange("c (b m) k -> c b m k", m=MB),
                    in_=a2aB_out.rearrange("b c m k -> c b m k"))

                # on-chip transpose (c2 partitions -> m partitions), packed
                # as [m, b, g, kj, c] so each (b, g) slice is a contiguous
                # 128-wide stationary operand with rows (kj, c)
                mchunks = [(ztmpA, 0, 128), (ztmpA, 128, 128),
                           (ztmpB, 0, NC * MB)]
                ztr = []
                with tc.tile_pool(name="ps2tp", bufs=6,
                                  space="PSUM") as ps2tp:
                    for mc, (src, off, mcnt) in enumerate(mchunks):
                        t = s2.tile([128, 4, NG, 4, 32], mybir.dt.bfloat16,
                                    tag=f"ztr{mc}")
                        nc.vector.memset(t[:, :, NG-1, 2:4, :], 0.0)
                        for kj in range(KC):
                            # transpose c2->m via a regular matmul against
                            # the identity (runs warm, unlike transpose-mode)
                            pt = ps2tp.tile([128, 128], mybir.dt.float32,
                                            tag="pt")
                            nc.tensor.matmul(
                                out=pt[:mcnt], lhsT=src[:, off:off+mcnt, kj],
                                rhs=ident[:], start=True, stop=True)
                            nc.vector.tensor_copy(
                                out=t[:mcnt, :, kj // 4, kj % 4, :],
                                in_=pt[:mcnt])
                        ztr.append(t)

                # comp 0 (s): srl rows (b=0) w/ Cre, sim rows (b=1) w/ Cim
                # comp 1 (t): trl rows (b=3) w/ Cre, tim rows (b=2) w/ Cim
                comp_seq = [((0, 0), (1, 1)), ((3, 0), (2, 1))]
                with tc.tile_pool(name="ps2po", bufs=4,
                                  space="PSUM") as ps2po:
                    for comp in range(2):
                        for g in range(NG):
                            po0 = ps2po.tile([128, 360], mybir.dt.float32,
                                             tag="po0")
                            po1 = ps2po.tile([128, 360], mybir.dt.float32,
                                             tag="po1")
                            i = 0
                            for (b, reim) in comp_seq[comp]:
                                for mc, (src, off, mcnt) in enumerate(mchunks):
                                    lhsT = ztr[mc][:mcnt, b, g]
                                    nc.tensor.matmul(
                                        out=po0[:], lhsT=lhsT,
                                        rhs=ct[:mcnt, reim, mc, 0:360],
                                        start=(i == 0), stop=(i == 5))
                                    nc.tensor.matmul(
                                        out=po1[:], lhsT=lhsT,
                                        rhs=ct[:mcnt, reim, mc, 360:720],
                                        start=(i == 0), stop=(i == 5))
                                    i += 1
                            osb = ob.tile([128, N], mybir.dt.float32,
                                          tag="osb")
                            nc.vector.tensor_copy(out=osb[:, 0:360],
                                                  in_=po0[:])
                            nc.vector.tensor_copy(out=osb[:, 360:720],
                                                  in_=po1[:])
                            # psum rows are (kj, c); alternate HWDGE rings
                            eng = nc.sync if (g % 2 == 0) else nc.scalar
                            eng.dma_start(
                                out=outsh[comp].rearrange(
                                    "c k n -> k c n")[g*4:(g+1)*4],
                                in_=osb[:],
                            )
    nc.compile()
    return nc


def _m_perm():
    """Row order of the m axis as seen by stage 2 (A-half then B-half)."""
    perm = [mb * MC + ml for mb in range(NC) for ml in range(MA)]
    perm += [mb * MC + ml for mb in range(NC) for ml in range(MA, MC)]
    return np.array(perm)


def _host_prep(x_re, x_im, d0, d1):
    xr0, xr1 = x_re[0, :, 0], x_re[0, :, 1]   # (32, L, M)
    xi0, xi1 = x_im[0, :, 0], x_im[0, :, 1]

    def mkx(blocks):
        x = np.concatenate(blocks, axis=0)            # (128, L, M)
        x = np.transpose(x, (2, 1, 0))                # (M, L, 128)
        xp = np.zeros((MP, LP, 128), BF16)
        xp[:M, :L] = x
        return xp
    X1 = mkx([xr0, xi0, -xi1, -xr1])
    X2 = mkx([-xi1, xr1, xr0, -xi0])
    # xsh[core][p, ml, which, lc, c'] = X{which}[core*MC+ml, lc*128+p, c']
    xv = np.stack([X1, X2], axis=1)                   # (MP, 2, LP, 128)
    xv = xv.reshape(NC, MC, 2, LCH, 128, 128)         # (i, ml, w, lc, p, c)
    xv = np.ascontiguousarray(xv.transpose(0, 4, 1, 2, 3, 5))

    def mkd(d):
        dp = np.zeros((MP, LP, KPP), BF16)
        dp[:M, :L, :K] = np.transpose(d, (0, 2, 1))
        return dp
    D0, D1 = mkd(d0), mkd(d1)
    dv = np.stack([D0, D1], axis=1)                   # (MP, 2, LP, KPP)
    dv = dv.reshape(NC, MC, 2, LCH, 128, KPP)
    dv = np.ascontiguousarray(dv.transpose(0, 4, 1, 2, 3, 5))

    m = np.arange(MP, dtype=np.float64)[:, None]
    n = np.arange(N, dtype=np.float64)[None, :]
    th = 2.0 * np.pi * (m * n) / N
    w = np.full((MP, 1), 2.0); w[0] = 1.0; w[360] = 1.0; w[361:] = 0.0
    Cre = (w * np.cos(th)).astype(np.float32)
    Cim = (-w * np.sin(th)).astype(np.float32)
    Cim[0] = 0.0; Cim[360] = 0.0; Cim[361:] = 0.0
    cv = np.stack([Cre, Cim], axis=1)                 # (MP, 2, N)
    cv = cv[_m_perm()]                                # stage-2 m order
    cv = np.concatenate(
        [cv, np.zeros((LCH * 128 - MP, 2, N), np.float32)], axis=0)
    cv = cv.reshape(LCH, 128, 2, N)
    cv = np.ascontiguousarray(cv.transpose(1, 2, 0, 3)).astype(BF16)
    return xv, dv, cv


def kernel(x_re, x_im, d0, d1):
    if "nc" not in _CACHE:
        _CACHE["nc"] = _build()
    nc = _CACHE["nc"]

    xv, dv, cv = _host_prep(np.asarray(x_re), np.asarray(x_im),
                            np.asarray(d0), np.asarray(d1))
    in_maps = [{"xsh": xv[i], "dsh": dv[i], "ctab": cv} for i in range(NC)]
    res = run_bass_kernel_spmd(nc, in_maps, list(range(NC)))

    out = np.empty((B, C, 2, K, N), np.float32)
    for i in range(NC):
        k0 = i * KC
        k1 = min(K, k0 + KC)
        o = res.results[i]["outsh"]        # [2, C, KG, N]
        out[0, :, 0, k0:k1] = o[0, :, :k1-k0]
        out[0, :, 1, k0:k1] = o[1, :, :k1-k0]
    return out
